# revision 15
# baseline (speedup 1.0000x reference)
"""Multi-head attention Trainium2 kernel (B=4, T=2048, C=1024, H=16).

Sharding: 8 cores = 4 batches x 2 head-groups (8 heads each).
Each core computes, for its (batch b, head set Hc):
  KhT/QhT = W @ x^T   [512, 2048] (head dims on partitions; Wq pre-scaled
            by 1/sqrt(dk) on host), Vh = x @ Wv^T (+ ones column per head)
  per head-pair hp (heads 2hp, 2hp+1 at partition offsets 0/64):
    S^T strip = Kh @ Qh^T  (k=64 row-group pair on the PE)
    P = exp(S^T)*mask: cols [0:768) exp on ACT, mask-mul on DVE/GpSimd;
        cols [768:1024) fused Schraudolph+mask in ONE DVE op:
        bf16(P) ~= bitcast_bf16(int16(S*184.665 + Bmask16)) with
        Bmask16 = 16249 (keep) / -27000 (masked -> |P| < 3e-12).
    Yaug^T = [Vh|1]^T @ P^T accumulated in PSUM; row 64 = softmax sums
    normalize via partition-scatter reciprocal, software-pipelined into
    the next pair's strips
  partial = YaT^T @ Wf[:, Hc]^T -> [2048, 1024], host sums 2 partials + bf.

Scheduling: Q/K projections run as a prologue (frees xq/xk SBUF); the V
projection and fc matmuls drip into the attention strip loop so the PE
stays continuously busy (p-state at 2.4 GHz); ACT does only exp + fc
psum copies. All matmuls bf16 with f32 PSUM accumulation.
"""

import numpy as np
import ml_dtypes

import concourse.bass as bass
import concourse.mybir as mybir
import concourse.tile as tile
from concourse import bacc
from concourse.bass_utils import run_bass_kernel_spmd

B, T, C, H = 4, 2048, 1024, 16
DK = C // H            # 64
GH = H // 2            # 8 heads per core
HD = GH * DK           # 512 head-dims per core
P = 128
KS = T // P            # 16 k-strips
NQA = 512              # q-chunk width
NQQ = T // NQA         # 4 q-chunks
NHP = GH // 2          # 4 head-pairs
NCORES = 8
SC = 256               # Schraudolph columns per strip (head B q[256:512))
K16 = 184.6650390625   # 2^7 / ln 2
B16 = 16249            # schraudolph bias (keep)
BM_MASKED = -27000     # schraudolph bias (masked)
DLY = 8                # PV trails exp by DLY strips
BF = mybir.dt.bfloat16
F32 = mybir.dt.float32
I16 = mybir.dt.int16
AF = mybir.ActivationFunctionType
ALU = mybir.AluOpType

LAST_RESULTS = None
_NC_CACHE = None


def build_bass():
    nc = bacc.Bacc()

    xqT_d = nc.dram_tensor("xqT", [C, T], BF, kind="ExternalInput")
    xkT_d = nc.dram_tensor("xkT", [C, T], BF, kind="ExternalInput")
    xvT_d = nc.dram_tensor("xvT", [C, T], BF, kind="ExternalInput")
    wqT_d = nc.dram_tensor("wqT", [C, HD], BF, kind="ExternalInput")
    wkT_d = nc.dram_tensor("wkT", [C, HD], BF, kind="ExternalInput")
    wvT_d = nc.dram_tensor("wvT", [C, HD], BF, kind="ExternalInput")
    wfT_d = nc.dram_tensor("wfT", [HD, C], BF, kind="ExternalInput")
    bq_d = nc.dram_tensor("bq", [P, HD // P], F32, kind="ExternalInput")
    bk_d = nc.dram_tensor("bk", [P, HD // P], F32, kind="ExternalInput")
    bvt_d = nc.dram_tensor("bvt", [P, HD], F32, kind="ExternalInput")
    maskT_d = nc.dram_tensor("maskT", [T, T], BF, kind="ExternalInput")
    bm16_d = nc.dram_tensor("bm16", [T, NQQ * SC], I16, kind="ExternalInput")
    out_d = nc.dram_tensor("out", [T, C], F32, kind="ExternalOutput")

    with tile.TileContext(nc) as tc:
        with (
            # sm first: the raw-AP partition scatter/broadcast DMAs in the
            # normalize chain only resolve at low SBUF addresses
            tc.tile_pool(name="sm", bufs=2) as small,
            tc.tile_pool(name="xs", bufs=11) as xpool,     # x^T strips [128,2048] bf16
            tc.tile_pool(name="ws", bufs=9) as wpool,      # W slices  [128,512]  bf16
            tc.tile_pool(name="wf", bufs=4) as wfpool,     # Wf slices [128,1024] bf16
            tc.tile_pool(name="qk", bufs=8) as qkpool,     # QhT/KhT   [128,2048] bf16
            tc.tile_pool(name="va", bufs=16) as vpool,     # Vaug      [128,520]  bf16
            tc.tile_pool(name="ya", bufs=4) as ypool,      # YaT       [128,2048] bf16
            tc.tile_pool(name="mk", bufs=19) as mpool,     # mask^T    [128,512]  bf16
            tc.tile_pool(name="bm", bufs=19) as bmpool,    # Bmask16   [128,256]  i16
            tc.tile_pool(name="pp", bufs=9) as ppool,     # P^T pairs [128,1024] bf16
            tc.tile_pool(name="ob", bufs=2) as opool,      # out stage [128,512]  f32
            tc.tile_pool(name="c1", bufs=1) as const_pool,
            tc.tile_pool(name="psS", bufs=2, space="PSUM") as psS,   # [128,1024]
            tc.tile_pool(name="psY", bufs=1, space="PSUM") as psY,   # [128,1024]
            tc.tile_pool(name="psB", bufs=2, space="PSUM") as psB,   # [128,512]
        ):
            # ---------------- constants ----------------
            bias_q = const_pool.tile([P, HD // P], F32, tag="bq", name="bq")
            nc.sync.dma_start(out=bias_q[:], in_=bq_d[:])
            bias_k = const_pool.tile([P, HD // P], F32, tag="bk", name="bk")
            nc.sync.dma_start(out=bias_k[:], in_=bk_d[:])
            bvt_sb = const_pool.tile([P, HD], F32, tag="bvt", name="bvt")
            nc.sync.dma_start(out=bvt_sb[:], in_=bvt_d[:])

            def load_xw(xT_d, wT_d):
                ws, xs = [], []
                for kc in range(C // P):
                    wt = wpool.tile([P, HD], BF, tag="ws", name="ws")
                    nc.sync.dma_start(out=wt[:], in_=wT_d[kc * P:(kc + 1) * P, :])
                    ws.append(wt)
                for kc in range(C // P):
                    xt = xpool.tile([P, T], BF, tag="xs", name="xs")
                    nc.sync.dma_start(out=xt[:], in_=xT_d[kc * P:(kc + 1) * P, :])
                    xs.append(xt)
                return ws, xs

            def proj_tiles(ws, xs, bias, tiles):
                # tiles: 4 x [128, T]; one [128,512] psum chunk at a time
                for mc in range(HD // P):
                    for ch in range(T // 512):
                        ps = psB.tile([P, 512], F32, tag="bg", name="bg")
                        for kc in range(C // P):
                            nc.tensor.matmul(
                                ps[:],
                                lhsT=ws[kc][:, mc * P:(mc + 1) * P],
                                rhs=xs[kc][:, ch * 512:(ch + 1) * 512],
                                start=(kc == 0),
                                stop=(kc == C // P - 1),
                            )
                        nc.vector.tensor_scalar_add(
                            tiles[mc][:, ch * 512:(ch + 1) * 512],
                            ps[:],
                            bias[:, mc:mc + 1],
                        )

            # ---------------- prologue: K then Q projections ----------------
            kws, kxs = load_xw(xkT_d, wkT_d)
            kT = [qkpool.tile([P, T], BF, tag="qk", name="kT") for _ in range(4)]
            proj_tiles(kws, kxs, bias_k, kT)
            qws, qxs = load_xw(xqT_d, wqT_d)
            qT = [qkpool.tile([P, T], BF, tag="qk", name="qT") for _ in range(4)]
            proj_tiles(qws, qxs, bias_q, qT)
            # V inputs (consumed by dripped V chunks during qq0)
            vws, vxs = load_xw(xvT_d, wvT_d)

            wf_sb = []
            for kc in range(HD // P):
                wt = wfpool.tile([P, C], BF, tag="wf", name="wf")
                nc.sync.dma_start(out=wt[:], in_=wfT_d[kc * P:(kc + 1) * P, :])
                wf_sb.append(wt)

            vts = []
            for ks in range(KS):
                vt = vpool.tile([P, GH * 65], BF, tag="va", name="va")
                nc.vector.memset(
                    vt.rearrange("p (h e) -> p h e", e=65)[:, :, 64:65], 1.0
                )
                vts.append(vt)

            mk_tiles = {}
            bm_tiles = {}

            def emit_mask_dma(qq, ks):
                mt = mpool.tile([P, NQA], BF, tag="mk", name="mk")
                nc.sync.dma_start(
                    out=mt[:],
                    in_=maskT_d[ks * P:(ks + 1) * P, qq * NQA:(qq + 1) * NQA],
                )
                mk_tiles[(qq, ks)] = mt
                bt = bmpool.tile([P, SC], I16, tag="bm", name="bm")
                nc.sync.dma_start(
                    out=bt[:],
                    in_=bm16_d[ks * P:(ks + 1) * P, qq * SC:(qq + 1) * SC],
                )
                bm_tiles[(qq, ks)] = bt

            for ks in range(KS):
                emit_mask_dma(0, ks)

            yaT = [ypool.tile([P, T], BF, tag="ya", name="ya") for _ in range(4)]

            # ---------------- dripped work ----------------
            def emit_v_chunk(ks):
                ps = psB.tile([P, HD], F32, tag="bg", name="bg")
                for kc in range(C // P):
                    nc.tensor.matmul(
                        ps[:],
                        lhsT=vxs[kc][:, ks * P:(ks + 1) * P],
                        rhs=vws[kc][:],
                        start=(kc == 0),
                        stop=(kc == C // P - 1),
                    )
                nc.vector.tensor_add(
                    vts[ks].rearrange("p (h e) -> p h e", e=65)[:, :, 0:64],
                    ps.rearrange("p (h d) -> p h d", d=DK),
                    bvt_sb.rearrange("p (h d) -> p h d", d=DK),
                )

            fc_count = [0]

            def emit_fc_chunk(mc, nn):
                ps = psB.tile([P, 512], F32, tag="bg", name="bg")
                for kc in range(HD // P):
                    nc.tensor.matmul(
                        ps[:],
                        lhsT=yaT[kc][:, mc * P:(mc + 1) * P],
                        rhs=wf_sb[kc][:, nn * 512:(nn + 1) * 512],
                        start=(kc == 0),
                        stop=(kc == HD // P - 1),
                    )
                ot = opool.tile([P, 512], F32, tag="ob", name="ob")
                nc.scalar.activation(ot[:], ps[:], AF.Copy)
                fc_count[0] += 1
                nc.sync.dma_start(
                    out=out_d[mc * P:(mc + 1) * P, nn * 512:(nn + 1) * 512],
                    in_=ot[:],
                )

            # ---------------- normalize (pipelined into next pair) --------
            # softmax sums live in yp row 64 (cols 0:512 head A, 512:1024
            # head B). Per head: copy sums row to SBUF (ACT), partition-
            # scatter [1,512]->[128,4] via DMA so the reciprocal uses all
            # lanes, DMA back, broadcast to [64,512], scale rows into yaT.
            def normalize_steps(hp, qq, yp):
                steps = []
                NW = NQA // P  # 4
                for hh in range(2):
                    srow = small.tile([1, NQA], F32, tag=f"rc{hh}", name="rc")
                    spread = small.tile([P, NW], F32, tag=f"sp{hh}", name="sp")
                    sprd_r = small.tile([P, NW], F32, tag=f"sr{hh}", name="sr")
                    rrow = small.tile([1, NQA], F32, tag=f"rr{hh}", name="rr")
                    rb = small.tile([DK, NQA], F32, tag=f"rb{hh}", name="rb")

                    def s1(srow=srow, hh=hh):
                        nc.scalar.activation(
                            srow[:], yp[64:65, hh * NQA:(hh + 1) * NQA], AF.Copy
                        )

                    def s2(srow=srow, spread=spread, sprd_r=sprd_r):
                        nc.sync.dma_start(
                            out=spread[:],
                            in_=bass.AP(tensor=srow.tensor, offset=srow.offset,
                                        ap=[[1, 1], [NW, P], [1, NW]]),
                        )
                        nc.vector.reciprocal(sprd_r[:], spread[:])

                    def s3(sprd_r=sprd_r, rrow=rrow):
                        nc.sync.dma_start(
                            out=bass.AP(tensor=rrow.tensor, offset=rrow.offset,
                                        ap=[[1, 1], [NW, P], [1, NW]]),
                            in_=sprd_r[:],
                        )

                    def s4(rrow=rrow, rb=rb):
                        nc.sync.dma_start(
                            out=rb[:],
                            in_=bass.AP(tensor=rrow.tensor, offset=rrow.offset,
                                        ap=[[1, 1], [0, DK], [1, NQA]]),
                        )

                    def s5(rb=rb, hh=hh):
                        nc.vector.tensor_mul(
                            yaT[hp][hh * DK:(hh + 1) * DK,
                                    qq * NQA:(qq + 1) * NQA],
                            yp[0:DK, hh * NQA:(hh + 1) * NQA],
                            rb[:],
                        )

                    steps += [s1, s2, s3, s4, s5]
                # interleave the two heads' chains: A1 B1 A2 B2 ...
                return [steps[i + 5 * h] for i in range(5) for h in range(2)]

            # ---------------- attention main loop ----------------
            pending_norm = []
            fc_queue = []
            fc_pending = []
            v_next = 0

            for qq in range(NQQ):
                for hp in range(NHP):
                    if hp == 1 and fc_pending:
                        fc_queue.extend(fc_pending)
                        fc_pending = []
                    kt = kT[hp]
                    qt = qT[hp]
                    box = {}
                    pend_mul = []   # (pt, mk) mask-muls pipelined 1 strip back

                    def flush_mul():
                        for pt_, mk_ in pend_mul:
                            nc.vector.tensor_mul(
                                pt_[:, 0:NQA], pt_[:, 0:NQA], mk_[:]
                            )
                            nc.gpsimd.tensor_mul(
                                pt_[:, NQA:2 * NQA - SC],
                                pt_[:, NQA:2 * NQA - SC],
                                mk_[:, 0:NQA - SC],
                            )
                        pend_mul.clear()

                    def emit_pv(ks, box=box, hp=hp):
                        pt = box.pop(ks)
                        yp = box["yp"]
                        for hh in range(2):
                            h = 2 * hp + hh
                            nc.tensor.matmul(
                                yp[0:65, hh * NQA:(hh + 1) * NQA],
                                lhsT=vts[ks][:, h * 65:(h + 1) * 65],
                                rhs=pt[:, hh * NQA:(hh + 1) * NQA],
                                start=(ks == 0),
                                stop=(ks == KS - 1),
                                skip_group_check=True,
                            )

                    for ks in range(KS):
                        # S-pair
                        sps = psS.tile([P, 2 * NQA], F32, tag="mm", name="mm")
                        for hh in range(2):
                            po = hh * DK
                            nc.tensor.matmul(
                                sps[:, hh * NQA:(hh + 1) * NQA],
                                lhsT=kt[po:po + DK, ks * P:(ks + 1) * P],
                                rhs=qt[po:po + DK, qq * NQA:(qq + 1) * NQA],
                                start=True,
                                stop=True,
                            )
                        # dripped PE work
                        if qq == 0 and v_next < KS:
                            emit_v_chunk(v_next)
                            v_next += 1
                        if fc_queue and ks % 4 == 1:
                            mc, nn = fc_queue.pop(0)
                            emit_fc_chunk(mc, nn)

                        # exp + schraudolph + mask
                        mk = mk_tiles[(qq, ks)]
                        bm = bm_tiles[(qq, ks)]
                        pt = ppool.tile([P, 2 * NQA], BF, tag="pp", name="pp")
                        nc.scalar.activation(
                            pt[:, 0:2 * NQA - SC], sps[:, 0:2 * NQA - SC], AF.Exp
                        )
                        nc.vector.scalar_tensor_tensor(
                            out=pt[:, 2 * NQA - SC:].bitcast(I16),
                            in0=sps[:, 2 * NQA - SC:],
                            scalar=K16,
                            in1=bm[:],
                            op0=ALU.mult,
                            op1=ALU.add,
                        )
                        flush_mul()
                        pend_mul.append((pt, mk))
                        box[ks] = pt

                        # normalize drip (prev pair), 2 steps per strip
                        for _ in range(2):
                            if pending_norm:
                                pending_norm.pop(0)()
                        # mask prefetch for qq+1 during last pair
                        if qq + 1 < NQQ and hp == 3:
                            emit_mask_dma(qq + 1, ks)

                        if ks == DLY:
                            box["yp"] = psY.tile(
                                [P, 2 * NQA], F32, tag="acc", name="acc"
                            )
                        if ks >= DLY:
                            emit_pv(ks - DLY)
                    flush_mul()
                    for ks in range(KS - DLY, KS):
                        emit_pv(ks)

                    for s in pending_norm:
                        s()
                    pending_norm = normalize_steps(hp, qq, box["yp"])

                for mc in range(qq * (NQA // P), (qq + 1) * (NQA // P)):
                    for nn in range(C // 512):
                        fc_pending.append((mc, nn))

            # tail
            for s in pending_norm:
                s()
            for mc, nn in fc_pending + fc_queue:
                emit_fc_chunk(mc, nn)
    return nc


def shard_inputs(q, k, v, mask, Wq, bq, Wk, bk, Wv, bv, Wf, bf):
    """Build the 8 per-core input maps (host-side prep, numpy only)."""
    bfl = ml_dtypes.bfloat16
    s = 1.0 / np.sqrt(DK)
    q, k, v = (np.asarray(a, np.float32) for a in (q, k, v))
    mask = np.asarray(mask)
    Wq, bq, Wk, bk, Wv, bv, Wf, bf = (
        np.asarray(a, np.float32) for a in (Wq, bq, Wk, bk, Wv, bv, Wf, bf)
    )
    in_maps = []
    for c in range(NCORES):
        b_, g = divmod(c, 2)
        hd = slice(g * HD, (g + 1) * HD)
        maskT = (mask[b_] != 0).T          # [k, q]
        # Bmask16[k, qq*SC + j] for q = qq*NQA + (NQA-SC) + j
        bm16 = np.empty((T, NQQ * SC), np.int16)
        for qq in range(NQQ):
            cols = maskT[:, qq * NQA + NQA - SC:(qq + 1) * NQA]
            bm16[:, qq * SC:(qq + 1) * SC] = np.where(cols, B16, BM_MASKED)
        im = {
            "xqT": np.ascontiguousarray(q[b_].T.astype(bfl)),
            "xkT": np.ascontiguousarray(k[b_].T.astype(bfl)),
            "xvT": np.ascontiguousarray(v[b_].T.astype(bfl)),
            "wqT": np.ascontiguousarray((Wq[hd, :] * s).T.astype(bfl)),
            "wkT": np.ascontiguousarray(Wk[hd, :].T.astype(bfl)),
            "wvT": np.ascontiguousarray(Wv[hd, :].T.astype(bfl)),
            "wfT": np.ascontiguousarray(Wf[:, hd].T.astype(bfl)),
            "bq": np.ascontiguousarray((bq[hd] * s).reshape(HD // P, P).T),
            "bk": np.ascontiguousarray(bk[hd].reshape(HD // P, P).T),
            "bvt": np.ascontiguousarray(
                np.broadcast_to(bv[hd], (P, HD)).astype(np.float32)
            ),
            "maskT": np.ascontiguousarray(maskT.astype(np.float32).astype(bfl)),
            "bm16": np.ascontiguousarray(bm16),
        }
        in_maps.append(im)
    return in_maps


def _get_bass():
    global _NC_CACHE
    if _NC_CACHE is None:
        nc = build_bass()
        nc.finalize()
        _NC_CACHE = nc
    return _NC_CACHE


def kernel(q, k, v, mask, Wq, bq, Wk, bk, Wv, bv, Wf, bf):
    global LAST_RESULTS
    nc = _get_bass()
    in_maps = shard_inputs(q, k, v, mask, Wq, bq, Wk, bk, Wv, bv, Wf, bf)
    res = run_bass_kernel_spmd(nc, in_maps, core_ids=list(range(NCORES)))
    LAST_RESULTS = res
    bf32 = np.asarray(bf, np.float32)
    out = np.empty((B, T, C), np.float32)
    for b_ in range(B):
        out[b_] = (
            res.results[2 * b_]["out"]
            + res.results[2 * b_ + 1]["out"]
            + bf32[None, :]
        )
    return out


# revision 16
# speedup vs baseline: 1.0242x; 1.0242x over previous
"""Multi-head attention Trainium2 kernel (B=4, T=2048, C=1024, H=16).

Sharding: 8 cores = 4 batches x 2 head-groups (8 heads each).
Each core computes, for its (batch b, head set Hc):
  KhT/QhT = W @ x^T   [512, 2048] (head dims on partitions; Wq pre-scaled
            by 1/sqrt(dk) on host), Vh = x @ Wv^T (+ ones column per head)
  per head-pair hp (heads 2hp, 2hp+1 at partition offsets 0/64):
    S^T strip = Kh @ Qh^T  (k=64 row-group pair on the PE)
    P = exp(S^T)*mask: cols [0:768) exp on ACT, mask-mul on DVE/GpSimd;
        cols [768:1024) fused Schraudolph+mask in ONE DVE op:
        bf16(P) ~= bitcast_bf16(int16(S*184.665 + Bmask16)) with
        Bmask16 = 16249 (keep) / -27000 (masked -> |P| < 3e-12).
    Yaug^T = [Vh|1]^T @ P^T accumulated in PSUM; row 64 = softmax sums
    normalize via partition-scatter reciprocal, software-pipelined into
    the next pair's strips
  partial = YaT^T @ Wf[:, Hc]^T -> [2048, 1024], host sums 2 partials + bf.

Scheduling: Q/K projections run as a prologue (frees xq/xk SBUF); the V
projection and fc matmuls drip into the attention strip loop so the PE
stays continuously busy (p-state at 2.4 GHz); ACT does only exp + fc
psum copies. All matmuls bf16 with f32 PSUM accumulation.
"""

import numpy as np
import ml_dtypes

import concourse.bass as bass
import concourse.mybir as mybir
import concourse.tile as tile
from concourse import bacc
from concourse.bass_utils import run_bass_kernel_spmd

B, T, C, H = 4, 2048, 1024, 16
DK = C // H            # 64
GH = H // 2            # 8 heads per core
HD = GH * DK           # 512 head-dims per core
P = 128
KS = T // P            # 16 k-strips
NQA = 512              # q-chunk width
NQQ = T // NQA         # 4 q-chunks
NHP = GH // 2          # 4 head-pairs
NCORES = 8
SC = 256               # Schraudolph columns per strip (head B q[256:512))
K16 = 184.6650390625   # 2^7 / ln 2
B16 = 16249            # schraudolph bias (keep)
BM_MASKED = -27000     # schraudolph bias (masked)
DLY = 8                # PV trails exp by DLY strips
BF = mybir.dt.bfloat16
F32 = mybir.dt.float32
I16 = mybir.dt.int16
AF = mybir.ActivationFunctionType
ALU = mybir.AluOpType

LAST_RESULTS = None
_NC_CACHE = None


def build_bass():
    nc = bacc.Bacc()

    xqT_d = nc.dram_tensor("xqT", [C, T], BF, kind="ExternalInput")
    xkT_d = nc.dram_tensor("xkT", [C, T], BF, kind="ExternalInput")
    xvT_d = nc.dram_tensor("xvT", [C, T], BF, kind="ExternalInput")
    wqT_d = nc.dram_tensor("wqT", [C, HD], BF, kind="ExternalInput")
    wkT_d = nc.dram_tensor("wkT", [C, HD], BF, kind="ExternalInput")
    wvT_d = nc.dram_tensor("wvT", [C, HD], BF, kind="ExternalInput")
    wfT_d = nc.dram_tensor("wfT", [HD, C], BF, kind="ExternalInput")
    bq_d = nc.dram_tensor("bq", [P, HD // P], F32, kind="ExternalInput")
    bk_d = nc.dram_tensor("bk", [P, HD // P], F32, kind="ExternalInput")
    bvt_d = nc.dram_tensor("bvt", [P, HD], F32, kind="ExternalInput")
    maskT_d = nc.dram_tensor("maskT", [T, T], BF, kind="ExternalInput")
    bm16_d = nc.dram_tensor("bm16", [T, NQQ * SC], I16, kind="ExternalInput")
    out_d = nc.dram_tensor("out", [T, C], F32, kind="ExternalOutput")

    with tile.TileContext(nc) as tc:
        with (
            # sm first: the raw-AP partition scatter/broadcast DMAs in the
            # normalize chain only resolve at low SBUF addresses
            tc.tile_pool(name="sm", bufs=2) as small,
            tc.tile_pool(name="xs", bufs=11) as xpool,     # x^T strips [128,2048] bf16
            tc.tile_pool(name="ws", bufs=9) as wpool,      # W slices  [128,512]  bf16
            tc.tile_pool(name="wf", bufs=4) as wfpool,     # Wf slices [128,1024] bf16
            tc.tile_pool(name="qk", bufs=8) as qkpool,     # QhT/KhT   [128,2048] bf16
            tc.tile_pool(name="va", bufs=16) as vpool,     # Vaug      [128,520]  bf16
            tc.tile_pool(name="ya", bufs=4) as ypool,      # YaT       [128,2048] bf16
            tc.tile_pool(name="mk", bufs=19) as mpool,     # mask^T    [128,512]  bf16
            tc.tile_pool(name="bm", bufs=19) as bmpool,    # Bmask16   [128,256]  i16
            tc.tile_pool(name="pp", bufs=9) as ppool,     # P^T pairs [128,1024] bf16
            tc.tile_pool(name="ob", bufs=2) as opool,      # out stage [128,512]  f32
            tc.tile_pool(name="c1", bufs=1) as const_pool,
            tc.tile_pool(name="psS", bufs=2, space="PSUM") as psS,   # [128,1024]
            tc.tile_pool(name="psY", bufs=1, space="PSUM") as psY,   # [128,1024]
            tc.tile_pool(name="psB", bufs=2, space="PSUM") as psB,   # [128,512]
        ):
            # ---------------- constants ----------------
            bias_q = const_pool.tile([P, HD // P], F32, tag="bq", name="bq")
            nc.sync.dma_start(out=bias_q[:], in_=bq_d[:])
            bias_k = const_pool.tile([P, HD // P], F32, tag="bk", name="bk")
            nc.sync.dma_start(out=bias_k[:], in_=bk_d[:])
            bvt_sb = const_pool.tile([P, HD], F32, tag="bvt", name="bvt")
            nc.sync.dma_start(out=bvt_sb[:], in_=bvt_d[:])

            def load_xw(xT_d, wT_d):
                ws, xs = [], []
                for kc in range(C // P):
                    wt = wpool.tile([P, HD], BF, tag="ws", name="ws")
                    nc.sync.dma_start(out=wt[:], in_=wT_d[kc * P:(kc + 1) * P, :])
                    ws.append(wt)
                for kc in range(C // P):
                    xt = xpool.tile([P, T], BF, tag="xs", name="xs")
                    nc.sync.dma_start(out=xt[:], in_=xT_d[kc * P:(kc + 1) * P, :])
                    xs.append(xt)
                return ws, xs

            def proj_tiles(ws, xs, bias, tiles):
                # tiles: 4 x [128, T]; one [128,512] psum chunk at a time
                for mc in range(HD // P):
                    for ch in range(T // 512):
                        ps = psB.tile([P, 512], F32, tag="bg", name="bg")
                        for kc in range(C // P):
                            nc.tensor.matmul(
                                ps[:],
                                lhsT=ws[kc][:, mc * P:(mc + 1) * P],
                                rhs=xs[kc][:, ch * 512:(ch + 1) * 512],
                                start=(kc == 0),
                                stop=(kc == C // P - 1),
                            )
                        nc.vector.tensor_scalar_add(
                            tiles[mc][:, ch * 512:(ch + 1) * 512],
                            ps[:],
                            bias[:, mc:mc + 1],
                        )

            # ---------------- prologue: K then Q projections ----------------
            kws, kxs = load_xw(xkT_d, wkT_d)
            kT = [qkpool.tile([P, T], BF, tag="qk", name="kT") for _ in range(4)]
            proj_tiles(kws, kxs, bias_k, kT)
            qws, qxs = load_xw(xqT_d, wqT_d)
            qT = [qkpool.tile([P, T], BF, tag="qk", name="qT") for _ in range(4)]
            proj_tiles(qws, qxs, bias_q, qT)
            # V inputs (consumed by dripped V chunks during qq0)
            vws, vxs = load_xw(xvT_d, wvT_d)

            wf_sb = []
            for kc in range(HD // P):
                wt = wfpool.tile([P, C], BF, tag="wf", name="wf")
                nc.sync.dma_start(out=wt[:], in_=wfT_d[kc * P:(kc + 1) * P, :])
                wf_sb.append(wt)

            vts = []
            for ks in range(KS):
                vt = vpool.tile([P, GH * 65], BF, tag="va", name="va")
                nc.vector.memset(
                    vt.rearrange("p (h e) -> p h e", e=65)[:, :, 64:65], 1.0
                )
                vts.append(vt)

            mk_tiles = {}
            bm_tiles = {}

            def emit_mask_dma(qq, ks):
                mt = mpool.tile([P, NQA], BF, tag="mk", name="mk")
                nc.sync.dma_start(
                    out=mt[:],
                    in_=maskT_d[ks * P:(ks + 1) * P, qq * NQA:(qq + 1) * NQA],
                )
                mk_tiles[(qq, ks)] = mt
                bt = bmpool.tile([P, SC], I16, tag="bm", name="bm")
                nc.sync.dma_start(
                    out=bt[:],
                    in_=bm16_d[ks * P:(ks + 1) * P, qq * SC:(qq + 1) * SC],
                )
                bm_tiles[(qq, ks)] = bt

            for ks in range(KS):
                emit_mask_dma(0, ks)

            yaT = [ypool.tile([P, T], BF, tag="ya", name="ya") for _ in range(4)]

            # ---------------- dripped work ----------------
            def emit_v_chunk(ks):
                ps = psB.tile([P, HD], F32, tag="bg", name="bg")
                for kc in range(C // P):
                    nc.tensor.matmul(
                        ps[:],
                        lhsT=vxs[kc][:, ks * P:(ks + 1) * P],
                        rhs=vws[kc][:],
                        start=(kc == 0),
                        stop=(kc == C // P - 1),
                    )
                nc.vector.tensor_add(
                    vts[ks].rearrange("p (h e) -> p h e", e=65)[:, :, 0:64],
                    ps.rearrange("p (h d) -> p h d", d=DK),
                    bvt_sb.rearrange("p (h d) -> p h d", d=DK),
                )

            fc_count = [0]

            def emit_fc_chunk(mc, nn):
                ps = psB.tile([P, 512], F32, tag="bg", name="bg")
                for kc in range(HD // P):
                    nc.tensor.matmul(
                        ps[:],
                        lhsT=yaT[kc][:, mc * P:(mc + 1) * P],
                        rhs=wf_sb[kc][:, nn * 512:(nn + 1) * 512],
                        start=(kc == 0),
                        stop=(kc == HD // P - 1),
                    )
                ot = opool.tile([P, 512], F32, tag="ob", name="ob")
                nc.scalar.activation(ot[:], ps[:], AF.Copy)
                fc_count[0] += 1
                nc.sync.dma_start(
                    out=out_d[mc * P:(mc + 1) * P, nn * 512:(nn + 1) * 512],
                    in_=ot[:],
                )

            # ---------------- normalize (pipelined into next pair) --------
            # softmax sums live in yp row 64 (cols 0:512 head A, 512:1024
            # head B). Per head: copy sums row to SBUF (ACT), partition-
            # scatter [1,512]->[128,4] via DMA so the reciprocal uses all
            # lanes, DMA back, broadcast to [64,512], scale rows into yaT.
            def normalize_steps(hp, qq, yp):
                steps = []
                NW = NQA // P  # 4
                for hh in range(2):
                    srow = small.tile([1, NQA], F32, tag=f"rc{hh}", name="rc")
                    spread = small.tile([P, NW], F32, tag=f"sp{hh}", name="sp")
                    sprd_r = small.tile([P, NW], F32, tag=f"sr{hh}", name="sr")
                    rrow = small.tile([1, NQA], F32, tag=f"rr{hh}", name="rr")
                    rb = small.tile([DK, NQA], F32, tag=f"rb{hh}", name="rb")

                    def s1(srow=srow, hh=hh):
                        nc.scalar.activation(
                            srow[:], yp[64:65, hh * NQA:(hh + 1) * NQA], AF.Copy
                        )

                    def s2(srow=srow, spread=spread, sprd_r=sprd_r):
                        nc.sync.dma_start(
                            out=spread[:],
                            in_=bass.AP(tensor=srow.tensor, offset=srow.offset,
                                        ap=[[1, 1], [NW, P], [1, NW]]),
                        )
                        nc.vector.reciprocal(sprd_r[:], spread[:])

                    def s3(sprd_r=sprd_r, rrow=rrow):
                        nc.sync.dma_start(
                            out=bass.AP(tensor=rrow.tensor, offset=rrow.offset,
                                        ap=[[1, 1], [NW, P], [1, NW]]),
                            in_=sprd_r[:],
                        )

                    def s4(rrow=rrow, rb=rb):
                        nc.sync.dma_start(
                            out=rb[:],
                            in_=bass.AP(tensor=rrow.tensor, offset=rrow.offset,
                                        ap=[[1, 1], [0, DK], [1, NQA]]),
                        )

                    def s5(rb=rb, hh=hh):
                        nc.vector.tensor_mul(
                            yaT[hp][hh * DK:(hh + 1) * DK,
                                    qq * NQA:(qq + 1) * NQA],
                            yp[0:DK, hh * NQA:(hh + 1) * NQA],
                            rb[:],
                        )

                    steps += [s1, s2, s3, s4, s5]
                # interleave the two heads' chains: A1 B1 A2 B2 ...
                return [steps[i + 5 * h] for i in range(5) for h in range(2)]

            # ---------------- attention main loop ----------------
            pending_norm = []
            fc_queue = []
            fc_pending = []
            v_next = 0

            for qq in range(NQQ):
                for hp in range(NHP):
                    if hp == 1 and fc_pending:
                        fc_queue.extend(fc_pending)
                        fc_pending = []
                    kt = kT[hp]
                    qt = qT[hp]
                    box = {}
                    pend_mul = []   # (pt, mk) mask-muls pipelined 1 strip back

                    def flush_mul():
                        for pt_, mk_ in pend_mul:
                            nc.vector.tensor_mul(
                                pt_[:, 0:NQA], pt_[:, 0:NQA], mk_[:]
                            )
                            nc.vector.tensor_mul(
                                pt_[:, NQA:2 * NQA - SC],
                                pt_[:, NQA:2 * NQA - SC],
                                mk_[:, 0:NQA - SC],
                            )
                        pend_mul.clear()

                    def emit_pv(ks, box=box, hp=hp):
                        pt = box.pop(ks)
                        yp = box["yp"]
                        for hh in range(2):
                            h = 2 * hp + hh
                            nc.tensor.matmul(
                                yp[0:65, hh * NQA:(hh + 1) * NQA],
                                lhsT=vts[ks][:, h * 65:(h + 1) * 65],
                                rhs=pt[:, hh * NQA:(hh + 1) * NQA],
                                start=(ks == 0),
                                stop=(ks == KS - 1),
                                skip_group_check=True,
                            )

                    for ks in range(KS):
                        # S-pair
                        sps = psS.tile([P, 2 * NQA], F32, tag="mm", name="mm")
                        for hh in range(2):
                            po = hh * DK
                            nc.tensor.matmul(
                                sps[:, hh * NQA:(hh + 1) * NQA],
                                lhsT=kt[po:po + DK, ks * P:(ks + 1) * P],
                                rhs=qt[po:po + DK, qq * NQA:(qq + 1) * NQA],
                                start=True,
                                stop=True,
                            )
                        # dripped PE work
                        if qq == 0 and v_next < KS:
                            emit_v_chunk(v_next)
                            v_next += 1
                        if fc_queue and ks % 4 == 1:
                            mc, nn = fc_queue.pop(0)
                            emit_fc_chunk(mc, nn)

                        # exp + schraudolph + mask
                        mk = mk_tiles[(qq, ks)]
                        bm = bm_tiles[(qq, ks)]
                        pt = ppool.tile([P, 2 * NQA], BF, tag="pp", name="pp")
                        nc.scalar.activation(
                            pt[:, 0:2 * NQA - SC], sps[:, 0:2 * NQA - SC], AF.Exp
                        )
                        nc.vector.scalar_tensor_tensor(
                            out=pt[:, 2 * NQA - SC:].bitcast(I16),
                            in0=sps[:, 2 * NQA - SC:],
                            scalar=K16,
                            in1=bm[:],
                            op0=ALU.mult,
                            op1=ALU.add,
                        )
                        flush_mul()
                        pend_mul.append((pt, mk))
                        box[ks] = pt

                        # normalize drip (prev pair), 2 steps per strip
                        for _ in range(2):
                            if pending_norm:
                                pending_norm.pop(0)()
                        # mask prefetch for qq+1 during last pair
                        if qq + 1 < NQQ and hp == 3:
                            emit_mask_dma(qq + 1, ks)

                        if ks == DLY:
                            box["yp"] = psY.tile(
                                [P, 2 * NQA], F32, tag="acc", name="acc"
                            )
                        if ks >= DLY:
                            emit_pv(ks - DLY)
                    flush_mul()
                    for ks in range(KS - DLY, KS):
                        emit_pv(ks)

                    for s in pending_norm:
                        s()
                    pending_norm = normalize_steps(hp, qq, box["yp"])

                for mc in range(qq * (NQA // P), (qq + 1) * (NQA // P)):
                    for nn in range(C // 512):
                        fc_pending.append((mc, nn))

            # tail
            for s in pending_norm:
                s()
            for mc, nn in fc_pending + fc_queue:
                emit_fc_chunk(mc, nn)
    return nc


def shard_inputs(q, k, v, mask, Wq, bq, Wk, bk, Wv, bv, Wf, bf):
    """Build the 8 per-core input maps (host-side prep, numpy only)."""
    bfl = ml_dtypes.bfloat16
    s = 1.0 / np.sqrt(DK)
    q, k, v = (np.asarray(a, np.float32) for a in (q, k, v))
    mask = np.asarray(mask)
    Wq, bq, Wk, bk, Wv, bv, Wf, bf = (
        np.asarray(a, np.float32) for a in (Wq, bq, Wk, bk, Wv, bv, Wf, bf)
    )
    in_maps = []
    for c in range(NCORES):
        b_, g = divmod(c, 2)
        hd = slice(g * HD, (g + 1) * HD)
        maskT = (mask[b_] != 0).T          # [k, q]
        # Bmask16[k, qq*SC + j] for q = qq*NQA + (NQA-SC) + j
        bm16 = np.empty((T, NQQ * SC), np.int16)
        for qq in range(NQQ):
            cols = maskT[:, qq * NQA + NQA - SC:(qq + 1) * NQA]
            bm16[:, qq * SC:(qq + 1) * SC] = np.where(cols, B16, BM_MASKED)
        im = {
            "xqT": np.ascontiguousarray(q[b_].T.astype(bfl)),
            "xkT": np.ascontiguousarray(k[b_].T.astype(bfl)),
            "xvT": np.ascontiguousarray(v[b_].T.astype(bfl)),
            "wqT": np.ascontiguousarray((Wq[hd, :] * s).T.astype(bfl)),
            "wkT": np.ascontiguousarray(Wk[hd, :].T.astype(bfl)),
            "wvT": np.ascontiguousarray(Wv[hd, :].T.astype(bfl)),
            "wfT": np.ascontiguousarray(Wf[:, hd].T.astype(bfl)),
            "bq": np.ascontiguousarray((bq[hd] * s).reshape(HD // P, P).T),
            "bk": np.ascontiguousarray(bk[hd].reshape(HD // P, P).T),
            "bvt": np.ascontiguousarray(
                np.broadcast_to(bv[hd], (P, HD)).astype(np.float32)
            ),
            "maskT": np.ascontiguousarray(maskT.astype(np.float32).astype(bfl)),
            "bm16": np.ascontiguousarray(bm16),
        }
        in_maps.append(im)
    return in_maps


def _get_bass():
    global _NC_CACHE
    if _NC_CACHE is None:
        nc = build_bass()
        nc.finalize()
        _NC_CACHE = nc
    return _NC_CACHE


def kernel(q, k, v, mask, Wq, bq, Wk, bk, Wv, bv, Wf, bf):
    global LAST_RESULTS
    nc = _get_bass()
    in_maps = shard_inputs(q, k, v, mask, Wq, bq, Wk, bk, Wv, bv, Wf, bf)
    res = run_bass_kernel_spmd(nc, in_maps, core_ids=list(range(NCORES)))
    LAST_RESULTS = res
    bf32 = np.asarray(bf, np.float32)
    out = np.empty((B, T, C), np.float32)
    for b_ in range(B):
        out[b_] = (
            res.results[2 * b_]["out"]
            + res.results[2 * b_ + 1]["out"]
            + bf32[None, :]
        )
    return out


# revision 19
# speedup vs baseline: 1.2286x; 1.1996x over previous
"""Multi-head attention Trainium2 kernel (B=4, T=2048, C=1024, H=16).

Sharding: 8 cores = 4 batches x 2 head-groups (8 heads each).
Each core computes, for its (batch b, head set Hc):
  KhT/QhT = W @ x^T   [512, 2048] (head dims on partitions; Wq pre-scaled
            by 1/sqrt(dk) on host), Vh = x @ Wv^T (+ ones column per head)
  per head-pair hp (heads 2hp, 2hp+1 at partition offsets 0/64):
    S^T strip = Kh @ Qh^T  (k=64 row-group pair on the PE)
    P = exp(S^T)*mask: cols [0:768) exp on ACT, mask-mul on DVE/GpSimd;
        cols [768:1024) fused Schraudolph+mask in ONE DVE op:
        bf16(P) ~= bitcast_bf16(int16(S*184.665 + Bmask16)) with
        Bmask16 = 16249 (keep) / -27000 (masked -> |P| < 3e-12).
    Yaug^T = [Vh|1]^T @ P^T accumulated in PSUM; row 64 = softmax sums
    normalize via partition-scatter reciprocal, software-pipelined into
    the next pair's strips
  partial = YaT^T @ Wf[:, Hc]^T -> [2048, 1024], host sums 2 partials + bf.

Scheduling: Q/K projections run as a prologue (frees xq/xk SBUF); the V
projection and fc matmuls drip into the attention strip loop so the PE
stays continuously busy (p-state at 2.4 GHz); ACT does only exp + fc
psum copies. All matmuls bf16 with f32 PSUM accumulation.
"""

import numpy as np
import ml_dtypes

import concourse.bass as bass
import concourse.mybir as mybir
import concourse.tile as tile
from concourse import bacc
from concourse.bass_utils import run_bass_kernel_spmd

B, T, C, H = 4, 2048, 1024, 16
DK = C // H            # 64
GH = H // 2            # 8 heads per core
HD = GH * DK           # 512 head-dims per core
P = 128
KS = T // P            # 16 k-strips
NQA = 512              # q-chunk width
NQQ = T // NQA         # 4 q-chunks
NHP = GH // 2          # 4 head-pairs
NCORES = 8
SC = 256               # Schraudolph columns per strip (head B q[256:512))
K16 = 184.6650390625   # 2^7 / ln 2
B16 = 16249            # schraudolph bias (keep)
BM_MASKED = -27000     # schraudolph bias (masked)
DLY = 8                # PV trails exp by DLY strips
BF = mybir.dt.bfloat16
F32 = mybir.dt.float32
I16 = mybir.dt.int16
AF = mybir.ActivationFunctionType
ALU = mybir.AluOpType

LAST_RESULTS = None
_NC_CACHE = None


def build_bass():
    nc = bacc.Bacc()

    xqT_d = nc.dram_tensor("xqT", [C, T], BF, kind="ExternalInput")
    xkT_d = nc.dram_tensor("xkT", [C, T], BF, kind="ExternalInput")
    xvT_d = nc.dram_tensor("xvT", [C, T], BF, kind="ExternalInput")
    wqT_d = nc.dram_tensor("wqT", [C, HD], BF, kind="ExternalInput")
    wkT_d = nc.dram_tensor("wkT", [C, HD], BF, kind="ExternalInput")
    wvT_d = nc.dram_tensor("wvT", [C, HD], BF, kind="ExternalInput")
    wfT_d = nc.dram_tensor("wfT", [HD, C], BF, kind="ExternalInput")
    bq_d = nc.dram_tensor("bq", [P, HD // P], F32, kind="ExternalInput")
    bk_d = nc.dram_tensor("bk", [P, HD // P], F32, kind="ExternalInput")
    bvt_d = nc.dram_tensor("bvt", [P, HD], F32, kind="ExternalInput")
    maskT_d = nc.dram_tensor("maskT", [T, T], BF, kind="ExternalInput")
    bm16_d = nc.dram_tensor("bm16", [T, NQQ * SC], I16, kind="ExternalInput")
    out_d = nc.dram_tensor("out", [T, C], F32, kind="ExternalOutput")

    with tile.TileContext(nc) as tc:
        with (
            # sm first: the raw-AP partition scatter/broadcast DMAs in the
            # normalize chain only resolve at low SBUF addresses
            tc.tile_pool(name="sm", bufs=2) as small,
            tc.tile_pool(name="xs", bufs=11) as xpool,     # x^T strips [128,2048] bf16
            tc.tile_pool(name="ws", bufs=9) as wpool,      # W slices  [128,512]  bf16
            tc.tile_pool(name="wf", bufs=4) as wfpool,     # Wf slices [128,1024] bf16
            tc.tile_pool(name="qk", bufs=8) as qkpool,     # QhT/KhT   [128,2048] bf16
            tc.tile_pool(name="va", bufs=16) as vpool,     # Vaug      [128,520]  bf16
            tc.tile_pool(name="ya", bufs=4) as ypool,      # YaT       [128,2048] bf16
            tc.tile_pool(name="mk", bufs=19) as mpool,     # mask^T    [128,512]  bf16
            tc.tile_pool(name="bm", bufs=19) as bmpool,    # Bmask16   [128,256]  i16
            tc.tile_pool(name="pp", bufs=9) as ppool,     # P^T pairs [128,1024] bf16
            tc.tile_pool(name="ob", bufs=2) as opool,      # out stage [128,512]  f32
            tc.tile_pool(name="c1", bufs=1) as const_pool,
            tc.tile_pool(name="psS", bufs=2, space="PSUM") as psS,   # [128,1024]
            tc.tile_pool(name="psY", bufs=1, space="PSUM") as psY,   # [128,1024]
            tc.tile_pool(name="psB", bufs=2, space="PSUM") as psB,   # [128,512]
        ):
            # ---------------- constants ----------------
            bias_q = const_pool.tile([P, HD // P], F32, tag="bq", name="bq")
            nc.sync.dma_start(out=bias_q[:], in_=bq_d[:])
            bias_k = const_pool.tile([P, HD // P], F32, tag="bk", name="bk")
            nc.sync.dma_start(out=bias_k[:], in_=bk_d[:])
            bvt_sb = const_pool.tile([P, HD], F32, tag="bvt", name="bvt")
            nc.sync.dma_start(out=bvt_sb[:], in_=bvt_d[:])

            def load_xw(xT_d, wT_d):
                ws, xs = [], []
                for kc in range(C // P):
                    wt = wpool.tile([P, HD], BF, tag="ws", name="ws")
                    nc.sync.dma_start(out=wt[:], in_=wT_d[kc * P:(kc + 1) * P, :])
                    ws.append(wt)
                for kc in range(C // P):
                    xt = xpool.tile([P, T], BF, tag="xs", name="xs")
                    nc.sync.dma_start(out=xt[:], in_=xT_d[kc * P:(kc + 1) * P, :])
                    xs.append(xt)
                return ws, xs

            def proj_tiles(ws, xs, bias, tiles):
                # tiles: 4 x [128, T]; one [128,512] psum chunk at a time
                for mc in range(HD // P):
                    for ch in range(T // 512):
                        ps = psB.tile([P, 512], F32, tag="bg", name="bg")
                        for kc in range(C // P):
                            nc.tensor.matmul(
                                ps[:],
                                lhsT=ws[kc][:, mc * P:(mc + 1) * P],
                                rhs=xs[kc][:, ch * 512:(ch + 1) * 512],
                                start=(kc == 0),
                                stop=(kc == C // P - 1),
                            )
                        nc.vector.tensor_scalar_add(
                            tiles[mc][:, ch * 512:(ch + 1) * 512],
                            ps[:],
                            bias[:, mc:mc + 1],
                        )

            # ---------------- prologue: K then Q projections ----------------
            kws, kxs = load_xw(xkT_d, wkT_d)
            kT = [qkpool.tile([P, T], BF, tag="qk", name="kT") for _ in range(4)]
            proj_tiles(kws, kxs, bias_k, kT)
            qws, qxs = load_xw(xqT_d, wqT_d)
            qT = [qkpool.tile([P, T], BF, tag="qk", name="qT") for _ in range(4)]
            proj_tiles(qws, qxs, bias_q, qT)
            # V inputs (consumed by dripped V chunks during qq0)
            vws, vxs = load_xw(xvT_d, wvT_d)

            wf_sb = []
            for kc in range(HD // P):
                wt = wfpool.tile([P, C], BF, tag="wf", name="wf")
                nc.sync.dma_start(out=wt[:], in_=wfT_d[kc * P:(kc + 1) * P, :])
                wf_sb.append(wt)

            vts = []
            for ks in range(KS):
                vt = vpool.tile([P, GH * 65], BF, tag="va", name="va")
                nc.vector.memset(
                    vt.rearrange("p (h e) -> p h e", e=65)[:, :, 64:65], 1.0
                )
                vts.append(vt)

            mk_tiles = {}
            bm_tiles = {}

            def emit_mask_dma(qq, ks):
                mt = mpool.tile([P, NQA], BF, tag="mk", name="mk")
                nc.sync.dma_start(
                    out=mt[:],
                    in_=maskT_d[ks * P:(ks + 1) * P, qq * NQA:(qq + 1) * NQA],
                )
                mk_tiles[(qq, ks)] = mt
                bt = bmpool.tile([P, SC], I16, tag="bm", name="bm")
                nc.sync.dma_start(
                    out=bt[:],
                    in_=bm16_d[ks * P:(ks + 1) * P, qq * SC:(qq + 1) * SC],
                )
                bm_tiles[(qq, ks)] = bt

            for ks in range(KS):
                emit_mask_dma(0, ks)

            yaT = [ypool.tile([P, T], BF, tag="ya", name="ya") for _ in range(4)]

            # ---------------- dripped work ----------------
            def emit_v_chunk(ks):
                ps = psB.tile([P, HD], F32, tag="bg", name="bg")
                for kc in range(C // P):
                    nc.tensor.matmul(
                        ps[:],
                        lhsT=vxs[kc][:, ks * P:(ks + 1) * P],
                        rhs=vws[kc][:],
                        start=(kc == 0),
                        stop=(kc == C // P - 1),
                    )
                nc.vector.tensor_add(
                    vts[ks].rearrange("p (h e) -> p h e", e=65)[:, :, 0:64],
                    ps.rearrange("p (h d) -> p h d", d=DK),
                    bvt_sb.rearrange("p (h d) -> p h d", d=DK),
                )

            fc_count = [0]

            def emit_fc_chunk(mc, nn):
                ps = psB.tile([P, 512], F32, tag="bg", name="bg")
                for kc in range(HD // P):
                    nc.tensor.matmul(
                        ps[:],
                        lhsT=yaT[kc][:, mc * P:(mc + 1) * P],
                        rhs=wf_sb[kc][:, nn * 512:(nn + 1) * 512],
                        start=(kc == 0),
                        stop=(kc == HD // P - 1),
                    )
                ot = opool.tile([P, 512], F32, tag="ob", name="ob")
                if fc_count[0] % 2 == 0:
                    nc.scalar.activation(ot[:], ps[:], AF.Copy)
                else:
                    nc.vector.tensor_scalar_add(ot[:], ps[:], 0.0)
                fc_count[0] += 1
                nc.sync.dma_start(
                    out=out_d[mc * P:(mc + 1) * P, nn * 512:(nn + 1) * 512],
                    in_=ot[:],
                )

            # ---------------- normalize (pipelined into next pair) --------
            # When a pair's last PV lands, copy yp out of PSUM immediately
            # (ACT takes the sums row, GpSimd the Y rows as bf16) so the
            # single psY buffer frees for the next pair; then reciprocal via
            # DMA partition-scatter and scale into yaT, all dripped into the
            # following strips.
            def normalize_steps(hp, qq, yp):
                ycop = small.tile([DK, 2 * NQA], BF, tag="yc", name="yc",
                                  bufs=1)
                steps = []

                def c0():
                    nc.vector.tensor_scalar_add(ycop[:], yp[0:DK, :], 0.0)
                steps.append(c0)

                NW = NQA // P  # 4
                for hh in range(2):
                    srow = small.tile([1, NQA], BF, tag=f"rc{hh}", name="rc",
                                      bufs=1)
                    spread = small.tile([P, NW], BF, tag=f"sp{hh}", name="sp",
                                        bufs=1)
                    sprd_r = small.tile([P, NW], BF, tag=f"sr{hh}", name="sr",
                                        bufs=1)
                    rrow = small.tile([1, NQA], BF, tag=f"rr{hh}", name="rr",
                                      bufs=1)
                    rb = small.tile([DK, NQA], BF, tag=f"rb{hh}", name="rb",
                                    bufs=1)

                    def s1(srow=srow, hh=hh):
                        nc.scalar.activation(
                            srow[:], yp[64:65, hh * NQA:(hh + 1) * NQA],
                            AF.Copy,
                        )

                    def s2(srow=srow, spread=spread, sprd_r=sprd_r):
                        nc.sync.dma_start(
                            out=spread[:],
                            in_=bass.AP(tensor=srow.tensor, offset=srow.offset,
                                        ap=[[1, 1], [NW, P], [1, NW]]),
                        )
                        with nc.allow_low_precision(
                            reason="softmax sums reciprocal in bf16"
                        ):
                            nc.vector.reciprocal(sprd_r[:], spread[:])

                    def s3(sprd_r=sprd_r, rrow=rrow):
                        nc.sync.dma_start(
                            out=bass.AP(tensor=rrow.tensor, offset=rrow.offset,
                                        ap=[[1, 1], [NW, P], [1, NW]]),
                            in_=sprd_r[:],
                        )

                    def s4(rrow=rrow, rb=rb):
                        nc.sync.dma_start(
                            out=rb[:],
                            in_=bass.AP(tensor=rrow.tensor, offset=rrow.offset,
                                        ap=[[1, 1], [0, DK], [1, NQA]]),
                        )

                    def s5(rb=rb, hh=hh):
                        nc.gpsimd.tensor_mul(
                            yaT[hp][hh * DK:(hh + 1) * DK,
                                    qq * NQA:(qq + 1) * NQA],
                            ycop[:, hh * NQA:(hh + 1) * NQA],
                            rb[:],
                        )

                    steps += [s1, s2, s3, s4, s5]
                # interleave: c0, A1, B1, A2, B2, ...
                return [steps[0]] + [steps[1 + i + 5 * h]
                                     for i in range(5) for h in range(2)]

            # ---------------- attention main loop ----------------
            pending_norm = []
            fc_queue = []
            fc_pending = []
            pend_mul = []
            pv_fifo = []
            v_next = 0

            def flush_mul():
                for pt_, mk_ in pend_mul:
                    nc.vector.tensor_mul(pt_[:, 0:NQA], pt_[:, 0:NQA], mk_[:])
                    nc.gpsimd.tensor_mul(
                        pt_[:, NQA:2 * NQA - SC],
                        pt_[:, NQA:2 * NQA - SC],
                        mk_[:, 0:NQA - SC],
                    )
                pend_mul.clear()

            def make_pv(box, hp, qq, ks):
                def em():
                    if ks == 0:
                        box["yp"] = psY.tile(
                            [P, 2 * NQA], F32, tag="acc", name="acc"
                        )
                    pt = box.pop(ks)
                    yp = box["yp"]
                    for hh in range(2):
                        h = 2 * hp + hh
                        nc.tensor.matmul(
                            yp[0:65, hh * NQA:(hh + 1) * NQA],
                            lhsT=vts[ks][:, h * 65:(h + 1) * 65],
                            rhs=pt[:, hh * NQA:(hh + 1) * NQA],
                            start=(ks == 0),
                            stop=(ks == KS - 1),
                            skip_group_check=True,
                        )
                    if ks == KS - 1:
                        pending_norm.extend(normalize_steps(hp, qq, yp))
                return em

            for qq in range(NQQ):
                for hp in range(NHP):
                    if hp == 1 and fc_pending:
                        fc_queue.extend(fc_pending)
                        fc_pending = []
                    kt = kT[hp]
                    qt = qT[hp]
                    box = {}

                    for ks in range(KS):
                        # S-pair
                        sps = psS.tile([P, 2 * NQA], F32, tag="mm", name="mm")
                        for hh in range(2):
                            po = hh * DK
                            nc.tensor.matmul(
                                sps[:, hh * NQA:(hh + 1) * NQA],
                                lhsT=kt[po:po + DK, ks * P:(ks + 1) * P],
                                rhs=qt[po:po + DK, qq * NQA:(qq + 1) * NQA],
                                start=True,
                                stop=True,
                            )
                        # exp + schraudolph
                        mk = mk_tiles[(qq, ks)]
                        bm = bm_tiles[(qq, ks)]
                        pt = ppool.tile([P, 2 * NQA], BF, tag="pp", name="pp")
                        nc.scalar.activation(
                            pt[:, 0:2 * NQA - SC], sps[:, 0:2 * NQA - SC],
                            AF.Exp,
                        )
                        nc.vector.scalar_tensor_tensor(
                            out=pt[:, 2 * NQA - SC:].bitcast(I16),
                            in0=sps[:, 2 * NQA - SC:],
                            scalar=K16,
                            in1=bm[:],
                            op0=ALU.mult,
                            op1=ALU.add,
                        )
                        # mask-muls for the previous strip
                        flush_mul()
                        pend_mul.append((pt, mk))
                        box[ks] = pt

                        # PV, delayed DLY strips, carried across pairs
                        pv_fifo.append(make_pv(box, hp, qq, ks))
                        if len(pv_fifo) > DLY:
                            pv_fifo.pop(0)()
                        # normalize drip (2 steps per strip)
                        for _ in range(2):
                            if pending_norm:
                                pending_norm.pop(0)()
                        # dripped PE work
                        if qq == 0 and v_next < KS:
                            emit_v_chunk(v_next)
                            v_next += 1
                        if fc_queue and ks % 4 == 1:
                            mc, nn = fc_queue.pop(0)
                            emit_fc_chunk(mc, nn)
                        # mask prefetch for qq+1 during last pair
                        if qq + 1 < NQQ and hp == 3:
                            emit_mask_dma(qq + 1, ks)

                for mc in range(qq * (NQA // P), (qq + 1) * (NQA // P)):
                    for nn in range(C // 512):
                        fc_pending.append((mc, nn))

            # tail
            flush_mul()
            for em in pv_fifo:
                em()
            pv_fifo = []
            while pending_norm:
                pending_norm.pop(0)()
            for mc, nn in fc_pending + fc_queue:
                emit_fc_chunk(mc, nn)
    return nc


def shard_inputs(q, k, v, mask, Wq, bq, Wk, bk, Wv, bv, Wf, bf):
    """Build the 8 per-core input maps (host-side prep, numpy only)."""
    bfl = ml_dtypes.bfloat16
    s = 1.0 / np.sqrt(DK)
    q, k, v = (np.asarray(a, np.float32) for a in (q, k, v))
    mask = np.asarray(mask)
    Wq, bq, Wk, bk, Wv, bv, Wf, bf = (
        np.asarray(a, np.float32) for a in (Wq, bq, Wk, bk, Wv, bv, Wf, bf)
    )
    in_maps = []
    for c in range(NCORES):
        b_, g = divmod(c, 2)
        hd = slice(g * HD, (g + 1) * HD)
        maskT = (mask[b_] != 0).T          # [k, q]
        # Bmask16[k, qq*SC + j] for q = qq*NQA + (NQA-SC) + j
        bm16 = np.empty((T, NQQ * SC), np.int16)
        for qq in range(NQQ):
            cols = maskT[:, qq * NQA + NQA - SC:(qq + 1) * NQA]
            bm16[:, qq * SC:(qq + 1) * SC] = np.where(cols, B16, BM_MASKED)
        im = {
            "xqT": np.ascontiguousarray(q[b_].T.astype(bfl)),
            "xkT": np.ascontiguousarray(k[b_].T.astype(bfl)),
            "xvT": np.ascontiguousarray(v[b_].T.astype(bfl)),
            "wqT": np.ascontiguousarray((Wq[hd, :] * s).T.astype(bfl)),
            "wkT": np.ascontiguousarray(Wk[hd, :].T.astype(bfl)),
            "wvT": np.ascontiguousarray(Wv[hd, :].T.astype(bfl)),
            "wfT": np.ascontiguousarray(Wf[:, hd].T.astype(bfl)),
            "bq": np.ascontiguousarray((bq[hd] * s).reshape(HD // P, P).T),
            "bk": np.ascontiguousarray(bk[hd].reshape(HD // P, P).T),
            "bvt": np.ascontiguousarray(
                np.broadcast_to(bv[hd], (P, HD)).astype(np.float32)
            ),
            "maskT": np.ascontiguousarray(maskT.astype(np.float32).astype(bfl)),
            "bm16": np.ascontiguousarray(bm16),
        }
        in_maps.append(im)
    return in_maps


def _get_bass():
    global _NC_CACHE
    if _NC_CACHE is None:
        nc = build_bass()
        nc.finalize()
        _NC_CACHE = nc
    return _NC_CACHE


def kernel(q, k, v, mask, Wq, bq, Wk, bk, Wv, bv, Wf, bf):
    global LAST_RESULTS
    nc = _get_bass()
    in_maps = shard_inputs(q, k, v, mask, Wq, bq, Wk, bk, Wv, bv, Wf, bf)
    res = run_bass_kernel_spmd(nc, in_maps, core_ids=list(range(NCORES)))
    LAST_RESULTS = res
    bf32 = np.asarray(bf, np.float32)
    out = np.empty((B, T, C), np.float32)
    for b_ in range(B):
        out[b_] = (
            res.results[2 * b_]["out"]
            + res.results[2 * b_ + 1]["out"]
            + bf32[None, :]
        )
    return out
